# revision 22
# baseline (speedup 1.0000x reference)
"""GAU (Gated Attention Unit) forward on 8 Trainium2 NeuronCores.

Data-parallel over batch: B=32 -> 4 batch elements per core, every core runs
the identical program on its batch shard with full (replicated) weights.

Per-core schedule (two weight-residency phases so SBUF fits):
  Phase 1 (uv_w.T qkv columns resident):
    LayerNorm -> PE-transpose xn -> qkv projection -> rope (PE half-swap) ->
    scoresT + Toeplitz bias -> relu^2 (fused DVE) -> attn @ v.
    Spills xnT and aT (attention output, transposed) to DRAM scratch.
  Phase 2 (uv_w.T u columns + o_w.T resident):
    u projection -> silu -> gate (u * a) -> output projection + residual.

All matmuls run in float32r (full-rate PE) with fp32 PSUM accumulation.
Weight DMAs are chunked and the first batch element consumes chunks as they
arrive (outer-k accumulation waves). SBUF pools are allocated bottom-up in
death-time order so phase-2 weight loads overlap the phase-1 tail. The
element loop is software-pipelined (LN/transpose of e+1 emitted between
projection and attention of e).
"""

import numpy as np
from contextlib import ExitStack

import concourse.bass as bass
import concourse.tile as tile
from concourse import bacc, mybir
from concourse.bass_utils import run_bass_kernel_spmd
from concourse.masks import make_identity

F32 = mybir.dt.float32
F32R = mybir.dt.float32r
AF = mybir.ActivationFunctionType
OP = mybir.AluOpType

B, T, H, E, S, L = 32, 512, 1024, 2048, 128, 512
NCORES = 8
BPC = B // NCORES          # batch elements per core
EPS = 1e-5
HC = H // 128              # 8 H-chunks
EC = E // 128              # 16 E-chunks
TC = T // 128              # 4 token chunks
NQKV = E + S               # 2176 qkv columns (v cols, then base cols)


def _emit(nc, with_vbias):
    x_d = nc.dram_tensor("x_in", [BPC, T, H], F32, kind="ExternalInput")
    wqkv_d = nc.dram_tensor("wqkv_in", [H, NQKV], F32R, kind="ExternalInput")
    wu_d = nc.dram_tensor("wu_in", [H, E], F32R, kind="ExternalInput")
    wo_d = nc.dram_tensor("wo_in", [E, H], F32R, kind="ExternalInput")
    biasT_d = nc.dram_tensor("biasT_in", [T, T], F32, kind="ExternalInput")
    ropeC_d = nc.dram_tensor("ropeC_in", [S, T], F32, kind="ExternalInput")
    ropeS_d = nc.dram_tensor("ropeS_in", [S, T], F32, kind="ExternalInput")
    gb_d = nc.dram_tensor("gb_in", [S, 4], F32, kind="ExternalInput")
    ubu_d = nc.dram_tensor("ubu_in", [128, EC], F32, kind="ExternalInput")
    ubb_d = nc.dram_tensor("ubb_in", [S, 1], F32, kind="ExternalInput")
    vb_d = nc.dram_tensor("vb_in", [1, E], F32R, kind="ExternalInput")
    y_d = nc.dram_tensor("y_out", [BPC, T, H], F32, kind="ExternalOutput")

    with tile.TileContext(nc) as tc, ExitStack() as ctx:
        consts = ctx.enter_context(tc.tile_pool(name="consts", bufs=1))
        holdp = ctx.enter_context(tc.tile_pool(name="holdp", bufs=1))
        dram = ctx.enter_context(tc.tile_pool(name="dram", bufs=1, space="DRAM"))
        xnT_spill = dram.tile([BPC, HC, 128, T], F32R)
        aT_spill = dram.tile([BPC, EC, 128, T], F32)

        # ================= PHASE 1 =================
        with ExitStack() as p1:
            # Pool allocation order = SBUF stack order = death-time order:
            # xp/xnp die after last LN, p1w after last qkv matmul, so the
            # phase-2 weight loads (which reuse this space) start early.
            xp = p1.enter_context(tc.tile_pool(name="xp", bufs=2))
            xnp = p1.enter_context(tc.tile_pool(name="xnp", bufs=2))
            p1w = p1.enter_context(tc.tile_pool(name="p1w", bufs=1))
            xnTp = p1.enter_context(tc.tile_pool(name="xnTp", bufs=1))
            vp = p1.enter_context(tc.tile_pool(name="vp", bufs=4))
            rw = p1.enter_context(tc.tile_pool(name="rw", bufs=2))
            ktp = p1.enter_context(tc.tile_pool(name="ktp", bufs=4))
            p1c = p1.enter_context(tc.tile_pool(name="p1c", bufs=1))
            atp = p1.enter_context(tc.tile_pool(name="atp", bufs=5))
            ps = p1.enter_context(tc.tile_pool(name="ps", bufs=4, space="PSUM"))
            psa = p1.enter_context(tc.tile_pool(name="psa", bufs=2, space="PSUM"))
            pstp = p1.enter_context(tc.tile_pool(name="pstp", bufs=2, space="PSUM"))

            eps_t = p1c.tile([128, 1], F32, tag="eps")
            nc.vector.memset(eps_t, EPS)

            def load_x_ln(e):
                xn_tiles = []
                for tci in range(TC):
                    xt = xp.tile([128, H], F32, tag="x")
                    nc.sync.dma_start(
                        out=xt, in_=x_d[e, tci * 128:(tci + 1) * 128, :])
                    st = rw.tile([128, 2, 6], F32, tag="bnst", bufs=4)
                    xv = xt[:].rearrange("p (g d) -> p g d", g=2)
                    nc.vector.bn_stats(out=st[:, 0, :], in_=xv[:, 0, :])
                    nc.vector.bn_stats(out=st[:, 1, :], in_=xv[:, 1, :])
                    mv = rw.tile([128, 2], F32, tag="bnmv", bufs=4)
                    nc.vector.bn_aggr(out=mv[:], in_=st[:])
                    std = rw.tile([128, 1], F32, tag="std", bufs=4)
                    nc.scalar.activation(
                        out=std[:], in_=mv[:, 1:2], func=AF.Sqrt,
                        bias=eps_t[:], scale=1.0)
                    rstd = rw.tile([128, 1], F32, tag="rstd", bufs=4)
                    nc.vector.reciprocal(out=rstd[:], in_=std[:])
                    xn = xnp.tile([128, H], F32R, tag="xn")
                    nc.vector.tensor_scalar(
                        out=xn[:], in0=xt[:], scalar1=mv[:, 0:1],
                        scalar2=rstd[:], op0=OP.subtract, op1=OP.mult)
                    xn_tiles.append(xn)
                return xn_tiles

            def transpose_xn(e, xn_tiles):
                """Per-tc transpose groups: xn(tc) is consumed right away."""
                if e == 0:
                    # element 0's xnT survives into phase 2 (no spill)
                    xnT = holdp.tile([128, HC, T], F32R, tag="xnT_hold",
                                     name="xnT_hold")
                else:
                    xnT = xnTp.tile([128, HC, T], F32R, tag="xnT")
                for tci in range(TC):
                    for hcg in range(2):
                        tps = pstp.tile([128, 4, 128], F32R, tag="tps",
                                      name="tps")
                        for j in range(4):
                            hc = hcg * 4 + j
                            nc.tensor.transpose(
                                tps[:, j, :],
                                xn_tiles[tci][:, hc * 128:(hc + 1) * 128],
                                ident[:])
                        nc.any.tensor_copy(
                            out=xnT[:, hcg * 4:(hcg + 1) * 4,
                                    tci * 128:(tci + 1) * 128],
                            in_=tps[:])
                if e != 0:
                    nc.sync.dma_start(
                        out=xnT_spill[e].rearrange("c p t -> p c t"),
                        in_=xnT[:])
                return xnT

            # ---- priority order: x(e0)+LN first, then consts, weights ----
            xn0 = load_x_ln(0)

            ident_f = p1c.tile([128, 128], F32, tag="ident_f")
            make_identity(nc, ident_f)
            ident = p1c.tile([128, 128], F32R, tag="ident")
            nc.vector.tensor_copy(out=ident[:], in_=ident_f[:])
            perm_f = p1c.tile([128, 128], F32, tag="perm_f")
            nc.gpsimd.memset(perm_f, 0.0)
            for base in (-64, 64):
                nc.gpsimd.affine_select(
                    out=perm_f, in_=perm_f, compare_op=OP.not_equal,
                    fill=1.0, base=base, pattern=[[-1, 128]],
                    channel_multiplier=1)
            perm = p1c.tile([128, 128], F32R, tag="perm")
            nc.vector.tensor_copy(out=perm[:], in_=perm_f[:])
            gb = p1c.tile([S, 4], F32, tag="gb")
            nc.sync.dma_start(out=gb, in_=gb_d[:])
            ubu = consts.tile([128, EC], F32, tag="ubu")
            nc.sync.dma_start(out=ubu, in_=ubu_d[:])
            ubb = p1c.tile([S, 1], F32, tag="ubb")
            nc.sync.dma_start(out=ubb, in_=ubb_d[:])
            if with_vbias:
                of = consts.tile([1, 128], F32, tag="ones_row_f")
                nc.vector.memset(of, 1.0)
                ones_row = consts.tile([1, 128], F32R, tag="ones_row")
                nc.vector.tensor_copy(out=ones_row[:], in_=of[:])
                vb_row = consts.tile([1, E], F32R, tag="vb_row")
                nc.sync.dma_start(out=vb_row, in_=vb_d[:])

            ropeC = p1c.tile([S, T], F32, tag="ropeC")
            nc.sync.dma_start(out=ropeC, in_=ropeC_d[:])
            ropeS = p1c.tile([S, T], F32, tag="ropeS")
            nc.sync.dma_start(out=ropeS, in_=ropeS_d[:])
            biasT = p1c.tile([128, TC, T], F32, tag="biasT")
            nc.sync.dma_start(
                out=biasT, in_=biasT_d[:].rearrange("(c p) i -> p c i", p=128))

            # qkv weights: per-chunk DMAs so elem-0 compute chases arrival
            wqkv = []
            for k in range(HC):
                wqk = p1w.tile([128, NQKV], F32R, tag=f"wqkv{k}", name="wqk")
                nc.sync.dma_start(
                    out=wqk, in_=wqkv_d[k * 128:(k + 1) * 128, :])
                wqkv.append(wqk)

            def rope_pre(bps):
                ubT = rw.tile([S, T], F32, tag="ubT")
                nc.scalar.activation(
                    out=ubT[:], in_=bps[:], func=AF.Silu, bias=ubb[:],
                    scale=1.0)
                pres = []
                for qi in (0, 1):
                    pre = rw.tile([S, T], F32R, tag="pre")
                    nc.vector.tensor_scalar(
                        out=pre[:], in0=ubT[:],
                        scalar1=gb[:, 2 * qi:2 * qi + 1],
                        scalar2=gb[:, 2 * qi + 1:2 * qi + 2],
                        op0=OP.mult, op1=OP.add)
                    pres.append(pre)
                return pres

            def rope_finish(pres):
                qkts = []
                for pre in pres:
                    sps = ps.tile([128, T], F32, tag="ps", name="sps")
                    nc.tensor.matmul(sps[:], perm[:], pre[:],
                                     start=True, stop=True)
                    t1 = rw.tile([S, T], F32, tag="t1")
                    nc.vector.tensor_tensor(
                        out=t1[:], in0=pre[:], in1=ropeC[:], op=OP.mult)
                    t2 = rw.tile([S, T], F32, tag="t2")
                    nc.vector.tensor_tensor(
                        out=t2[:], in0=sps[:], in1=ropeS[:], op=OP.mult)
                    qkt = rw.tile([S, T], F32R, tag="qkt", bufs=3)
                    nc.vector.tensor_tensor(
                        out=qkt[:], in0=t1[:], in1=t2[:], op=OP.add)
                    qkts.append(qkt)
                return qkts

            def base_proj(xnT):
                bps = ps.tile([128, T], F32, tag="ps", name="bps")
                for k in range(HC):
                    nc.tensor.matmul(
                        bps[:], wqkv[k][:, E:E + S], xnT[:, k, :],
                        start=(k == 0), stop=(k == HC - 1))
                return bps

            def qkv_chase(xnT):
                """Elem-0 path: base + v projections in 6-bank waves that
                consume weight chunks as they arrive."""
                v_tiles = [vp.tile([128, E], F32R, tag="v", name="v")
                           for _ in range(TC)]
                pairs = [(tci, fs) for tci in range(TC)
                         for fs in range(E // 512)]

                def fin(tci, fs, bank):
                    if with_vbias:
                        nc.tensor.matmul(
                            bank[:], ones_row[:],
                            vb_row[:, fs * 512:(fs + 1) * 512],
                            start=False, stop=True)
                    nc.scalar.activation(
                        out=v_tiles[tci][:, fs * 512:(fs + 1) * 512],
                        in_=bank[:], func=AF.Silu)

                bps = ps.tile([128, T], F32, tag="ps", name="bps")
                wave0 = pairs[:5]
                banks0 = {p: ps.tile([128, 512], F32, tag="ps", name="vbank")
                          for p in wave0}
                for k in range(HC):
                    nc.tensor.matmul(
                        bps[:], wqkv[k][:, E:E + S], xnT[:, k, :],
                        start=(k == 0), stop=(k == HC - 1))
                    for (tci, fs) in wave0:
                        nc.tensor.matmul(
                            banks0[(tci, fs)][:],
                            xnT[:, k, tci * 128:(tci + 1) * 128],
                            wqkv[k][:, fs * 512:(fs + 1) * 512],
                            start=(k == 0),
                            stop=(k == HC - 1) and not with_vbias)
                pres = rope_pre(bps)
                for (tci, fs) in wave0:
                    fin(tci, fs, banks0[(tci, fs)])
                qkts = None
                for wn, wi in enumerate(range(5, len(pairs), 6)):
                    wave = pairs[wi:wi + 6]
                    banks = {p: ps.tile([128, 512], F32, tag="ps",
                                        name="vbank") for p in wave}
                    for k in range(HC):
                        for (tci, fs) in wave:
                            nc.tensor.matmul(
                                banks[(tci, fs)][:],
                                xnT[:, k, tci * 128:(tci + 1) * 128],
                                wqkv[k][:, fs * 512:(fs + 1) * 512],
                                start=(k == 0),
                                stop=(k == HC - 1) and not with_vbias)
                    if wn == 0:
                        qkts = rope_finish(pres)
                    for (tci, fs) in wave:
                        fin(tci, fs, banks[(tci, fs)])
                return qkts, v_tiles

            def v_proj(xnT, outer_k, rope_mid=None):
                v_tiles = [vp.tile([128, E], F32R, tag="v", name="v")
                           for _ in range(TC)]
                pairs = [(tci, fs) for tci in range(TC)
                         for fs in range(E // 512)]

                def fin(tci, fs, bank):
                    if with_vbias:
                        nc.tensor.matmul(
                            bank[:], ones_row[:],
                            vb_row[:, fs * 512:(fs + 1) * 512],
                            start=False, stop=True)
                    nc.scalar.activation(
                        out=v_tiles[tci][:, fs * 512:(fs + 1) * 512],
                        in_=bank[:], func=AF.Silu)

                if outer_k:
                    for wi in range(0, len(pairs), 3):
                        wave = pairs[wi:wi + 3]
                        banks = {p: ps.tile([128, 512], F32, tag="ps",
                                            name="vbank") for p in wave}
                        for k in range(HC):
                            for (tci, fs) in wave:
                                nc.tensor.matmul(
                                    banks[(tci, fs)][:],
                                    xnT[:, k, tci * 128:(tci + 1) * 128],
                                    wqkv[k][:, fs * 512:(fs + 1) * 512],
                                    start=(k == 0),
                                    stop=(k == HC - 1) and not with_vbias)
                        for (tci, fs) in wave:
                            fin(tci, fs, banks[(tci, fs)])
                else:
                    for pi, (tci, fs) in enumerate(pairs):
                        bank = ps.tile([128, 512], F32, tag="ps", name="vbank")
                        for k in range(HC):
                            nc.tensor.matmul(
                                bank[:],
                                xnT[:, k, tci * 128:(tci + 1) * 128],
                                wqkv[k][:, fs * 512:(fs + 1) * 512],
                                start=(k == 0),
                                stop=(k == HC - 1) and not with_vbias)
                        fin(tci, fs, bank)
                        if pi == 3 and rope_mid is not None:
                            rope_mid()
                return v_tiles

            def attention(e, qT, kT, v_tiles):
                kern_tiles = []
                for jc in range(TC):
                    scps = ps.tile([128, T], F32, tag="ps", name="scps")
                    nc.tensor.matmul(
                        scps[:], kT[:, jc * 128:(jc + 1) * 128], qT[:],
                        start=True, stop=True)
                    tadd = rw.tile([128, T], F32, tag="tadd")
                    nc.vector.tensor_tensor(
                        out=tadd[:], in0=scps[:], in1=biasT[:, jc, :],
                        op=OP.add)
                    kt = ktp.tile([128, T], F32R, tag="kern")
                    # relu(t)^2 == max(t,0)*t, fused on DVE
                    nc.vector.scalar_tensor_tensor(
                        out=kt[:], in0=tadd[:], scalar=0.0, in1=tadd[:],
                        op0=OP.max, op1=OP.mult)
                    kern_tiles.append(kt)
                for ec in range(EC):
                    aps = psa.tile([128, T], F32, tag="psa", name="aps")
                    for jc in range(TC):
                        nc.tensor.matmul(
                            aps[:],
                            v_tiles[jc][:, ec * 128:(ec + 1) * 128],
                            kern_tiles[jc][:],
                            start=(jc == 0), stop=(jc == TC - 1))
                    at = atp.tile([128, T], F32, tag="aT")
                    nc.any.tensor_copy(out=at[:], in_=aps[:])
                    nc.sync.dma_start(out=aT_spill[e, ec], in_=at[:])

            xnT_hold = None
            for e in range(BPC):
                xn_e = xn0 if e == 0 else load_x_ln(e)
                xnT_e = transpose_xn(e, xn_e)
                if e == 0:
                    xnT_hold = xnT_e
                if e == 0:
                    qk_e, v_e = qkv_chase(xnT_e)
                else:
                    bps = base_proj(xnT_e)
                    pres = rope_pre(bps)
                    qk_box = []

                    def rope_mid(pres=pres, qk_box=qk_box):
                        qk_box.extend(rope_finish(pres))

                    v_e = v_proj(xnT_e, outer_k=False, rope_mid=rope_mid)
                    qk_e = qk_box
                attention(e, *qk_e, v_e)

        # ================= PHASE 2 =================
        with ExitStack() as p2:
            xnT2p = p2.enter_context(tc.tile_pool(name="xnT2p", bufs=1))
            p2w = p2.enter_context(tc.tile_pool(name="p2w", bufs=1))
            utp = p2.enter_context(tc.tile_pool(name="utp", bufs=1))
            at2p = p2.enter_context(tc.tile_pool(name="at2p", bufs=2))
            gtp = p2.enter_context(tc.tile_pool(name="gtp", bufs=EC))
            yp = p2.enter_context(tc.tile_pool(name="yp", bufs=2))
            xrp = p2.enter_context(tc.tile_pool(name="xrp", bufs=2))
            ps2 = p2.enter_context(tc.tile_pool(name="ps2", bufs=8,
                                                space="PSUM"))

            def load_xnT2(e):
                xnT2 = xnT2p.tile([128, HC, T], F32R, tag="xnT2")
                nc.sync.dma_start(
                    out=xnT2, in_=xnT_spill[e].rearrange("c p t -> p c t"))
                return xnT2

            wu = []
            for k in range(HC):
                wuk = p2w.tile([128, E], F32R, tag=f"wu{k}", name="wuk")
                nc.sync.dma_start(out=wuk, in_=wu_d[k * 128:(k + 1) * 128, :])
                wu.append(wuk)
            wo = [None] * EC

            def load_wo_chunk(ec):
                woc = p2w.tile([128, H], F32R, tag=f"wo{ec}", name="woc")
                nc.sync.dma_start(
                    out=woc, in_=wo_d[ec * 128:(ec + 1) * 128, :])
                wo[ec] = woc

            def gate_one(e, ec, bank):
                ut = utp.tile([128, T], F32, tag="uT")
                nc.scalar.activation(
                    out=ut[:], in_=bank[:], func=AF.Silu,
                    bias=ubu[:, ec:ec + 1], scale=1.0)
                at2 = at2p.tile([128, T], F32, tag="aT2")
                nc.sync.dma_start(out=at2, in_=aT_spill[e, ec])
                gt = gtp.tile([128, T], F32R, tag="gT")
                nc.vector.tensor_tensor(
                    out=gt[:], in0=ut[:], in1=at2[:], op=OP.mult)
                return gt

            def u_proj_gate(e, xnT2, outer_k):
                g_tiles = []
                if outer_k:
                    nwo = 0
                    for wi in range(0, EC, 6):
                        wave = list(range(wi, min(wi + 6, EC)))
                        banks = {ec: ps2.tile([128, T], F32, tag="ps",
                                              name="ubank") for ec in wave}
                        for k in range(HC):
                            for ec in wave:
                                nc.tensor.matmul(
                                    banks[ec][:],
                                    wu[k][:, ec * 128:(ec + 1) * 128],
                                    xnT2[:, k, :],
                                    start=(k == 0), stop=(k == HC - 1))
                        for ec in wave:
                            g_tiles.append(gate_one(e, ec, banks[ec]))
                            if nwo < EC:
                                load_wo_chunk(nwo)
                                nwo += 1
                    while nwo < EC:
                        load_wo_chunk(nwo)
                        nwo += 1
                else:
                    for ec in range(EC):
                        bank = ps2.tile([128, T], F32, tag="ps", name="ubank")
                        for k in range(HC):
                            nc.tensor.matmul(
                                bank[:], wu[k][:, ec * 128:(ec + 1) * 128],
                                xnT2[:, k, :],
                                start=(k == 0), stop=(k == HC - 1))
                        g_tiles.append(gate_one(e, ec, bank))
                return g_tiles

            def o_proj_tail(e, g_tiles):
                # last element: per-(tc,hs) accumulation with immediate
                # evacuation, so the kernel tail isn't one long serial chain
                pairs8 = [(tci, hs) for tci in range(TC)
                          for hs in range(H // 512)]
                for (tci, hs) in pairs8:
                    xr = xrp.tile([128, 512], F32, tag="xr")
                    nc.sync.dma_start(
                        out=xr,
                        in_=x_d[e, tci * 128:(tci + 1) * 128,
                                hs * 512:(hs + 1) * 512])
                    yps = ps2.tile([128, 512], F32, tag="ps", name="yps")
                    for ec in range(EC):
                        nc.tensor.matmul(
                            yps[:],
                            g_tiles[ec][:, tci * 128:(tci + 1) * 128],
                            wo[ec][:, hs * 512:(hs + 1) * 512],
                            start=(ec == 0), stop=(ec == EC - 1))
                    yt = yp.tile([128, 512], F32, tag="y")
                    nc.vector.tensor_tensor(
                        out=yt[:], in0=yps[:], in1=xr[:], op=OP.add)
                    nc.sync.dma_start(
                        out=y_d[e, tci * 128:(tci + 1) * 128,
                                hs * 512:(hs + 1) * 512],
                        in_=yt[:])

            def o_proj(e, g_tiles):
                # outer-ec accumulation into 8 banks: consumes each gT chunk
                # exactly once (frees it for the next element's gating) and
                # chases wo chunk arrival on the first element.
                pairs8 = [(tci, hs) for tci in range(TC)
                          for hs in range(H // 512)]
                banks = {p: ps2.tile([128, 512], F32, tag="ps", name="obank")
                         for p in pairs8}
                xrs = {}
                for (tci, hs) in pairs8:
                    xr = xrp.tile([128, 512], F32, tag="xr")
                    nc.sync.dma_start(
                        out=xr,
                        in_=x_d[e, tci * 128:(tci + 1) * 128,
                                hs * 512:(hs + 1) * 512])
                    xrs[(tci, hs)] = xr
                for ec in range(EC):
                    for (tci, hs) in pairs8:
                        nc.tensor.matmul(
                            banks[(tci, hs)][:],
                            g_tiles[ec][:, tci * 128:(tci + 1) * 128],
                            wo[ec][:, hs * 512:(hs + 1) * 512],
                            start=(ec == 0), stop=(ec == EC - 1))
                for (tci, hs) in pairs8:
                    yt = yp.tile([128, 512], F32, tag="y")
                    nc.vector.tensor_tensor(
                        out=yt[:], in0=banks[(tci, hs)][:], in1=xrs[(tci, hs)][:],
                        op=OP.add)
                    nc.sync.dma_start(
                        out=y_d[e, tci * 128:(tci + 1) * 128,
                                hs * 512:(hs + 1) * 512],
                        in_=yt[:])

            for e in range(BPC):
                xnT2 = xnT_hold if e == 0 else load_xnT2(e)
                g_tiles = u_proj_gate(e, xnT2, outer_k=(e == 0))
                if e == BPC - 1:
                    o_proj_tail(e, g_tiles)
                else:
                    o_proj(e, g_tiles)

    return nc


_BUILD_CACHE = {}


def _get_nc(with_vbias):
    key = bool(with_vbias)
    if key not in _BUILD_CACHE:
        nc = bacc.Bacc("TRN2", target_bir_lowering=False)
        _emit(nc, with_vbias)
        nc.compile()
        _BUILD_CACHE[key] = nc
    return _BUILD_CACHE[key]


def _rope_tables():
    """Rope sin/cos tables, computed with jax-on-cpu float32 ops exactly as
    the reference does (sin/cos of large fp32 arguments are implementation-
    sensitive, so matching op-for-op matters)."""
    import jax
    import jax.numpy as jnp

    cpu = jax.devices("cpu")[0]
    with jax.default_device(cpu):
        half = S // 2
        pos = jnp.arange(T, dtype=jnp.float32)
        inv_freq = 10000.0 ** (jnp.arange(half, dtype=jnp.float32) / half)
        sinusoid = pos[:, None] * inv_freq[None, :]          # [T, half]
        sin = np.asarray(jnp.sin(sinusoid)).astype(np.float32)
        cos = np.asarray(jnp.cos(sinusoid)).astype(np.float32)
    C = np.empty((S, T), np.float32)
    Sg = np.empty((S, T), np.float32)
    C[:half] = cos.T
    C[half:] = cos.T
    Sg[:half] = -sin.T   # q[s<64] = pre[s]*cos - pre[s+64]*sin
    Sg[half:] = sin.T    # q[s>=64] = pre[s]*cos + pre[s-64]*sin
    return C, Sg


def _host_prep(x, ln_w, ln_b, uv_w, uv_b, gamma, beta, w, o_w, o_b):
    w_eff = uv_w * ln_w[None, :]                 # fold ln scale into weights
    uvb_eff = uv_b + uv_w @ ln_b                 # fold ln shift into biases
    uv_wT = np.ascontiguousarray(w_eff.T)        # [H, 2E+S]
    wqkv = np.ascontiguousarray(uv_wT[:, E:])    # [H, E+S]
    wu = np.ascontiguousarray(uv_wT[:, :E])      # [H, E]
    wo = np.ascontiguousarray(o_w.T)             # [E, H]

    idx = np.arange(T)
    biasT = np.ascontiguousarray(w[idx[:, None] - idx[None, :] + (L - 1)])

    ropeC, ropeS = _rope_tables()

    inv_sqrt_s = np.float32(1.0 / np.sqrt(np.float32(S)))
    gb = np.stack([gamma[0] * inv_sqrt_s, beta[0] * inv_sqrt_s,
                   gamma[1], beta[1]], axis=1).astype(np.float32)

    ubu = np.ascontiguousarray(
        uvb_eff[:E].reshape(EC, 128).T).astype(np.float32)
    ubb = uvb_eff[2 * E:].reshape(S, 1).astype(np.float32)
    vb = uvb_eff[E:2 * E].reshape(1, E).astype(np.float32)
    return {
        "wqkv_in": wqkv, "wu_in": wu, "wo_in": wo, "biasT_in": biasT,
        "ropeC_in": ropeC, "ropeS_in": ropeS, "gb_in": gb,
        "ubu_in": ubu, "ubb_in": ubb, "vb_in": vb,
    }


def kernel(x, ln_w, ln_b, uv_w, uv_b, gamma, beta, w, o_w, o_b):
    x = np.ascontiguousarray(np.asarray(x, dtype=np.float32))
    args = [np.asarray(a, np.float32) for a in
            (ln_w, ln_b, uv_w, uv_b, gamma, beta, w, o_w, o_b)]
    ln_w, ln_b, uv_w, uv_b, gamma, beta, w, o_w, o_b = args

    shared = _host_prep(x, ln_w, ln_b, uv_w, uv_b, gamma, beta, w, o_w, o_b)
    with_vbias = bool(np.any(shared["vb_in"]))
    nc = _get_nc(with_vbias)

    in_maps = []
    for c in range(NCORES):
        m = dict(shared)
        m["x_in"] = np.ascontiguousarray(x[c * BPC:(c + 1) * BPC])
        in_maps.append(m)

    res = run_bass_kernel_spmd(nc, in_maps, core_ids=list(range(NCORES)))
    out = np.concatenate([r["y_out"] for r in res.results], axis=0)
    if np.any(o_b):
        out = out + o_b[None, None, :]
    return out


# revision 24
# speedup vs baseline: 250.7721x; 250.7721x over previous
"""GAU (Gated Attention Unit) forward on 8 Trainium2 NeuronCores.

Data-parallel over batch: B=32 -> 4 batch elements per core, every core runs
the identical program on its batch shard with full (replicated) weights.

Per-core schedule (two weight-residency phases so SBUF fits):
  Phase 1 (uv_w.T qkv columns resident):
    LayerNorm -> PE-transpose xn -> qkv projection -> rope (PE half-swap) ->
    scoresT + Toeplitz bias -> relu^2 (fused DVE) -> attn @ v.
    Spills xnT and aT (attention output, transposed) to DRAM scratch.
  Phase 2 (uv_w.T u columns + o_w.T resident):
    u projection -> silu -> gate (u * a) -> output projection + residual.

All matmuls run in float32r (full-rate PE) with fp32 PSUM accumulation.
Weight DMAs are chunked and the first batch element consumes chunks as they
arrive (outer-k accumulation waves). SBUF pools are allocated bottom-up in
death-time order so phase-2 weight loads overlap the phase-1 tail. The
element loop is software-pipelined (LN/transpose of e+1 emitted between
projection and attention of e).
"""

import numpy as np
from contextlib import ExitStack

import concourse.bass as bass
import concourse.tile as tile
from concourse import bacc, mybir
from concourse.bass_utils import run_bass_kernel_spmd
from concourse.masks import make_identity

F32 = mybir.dt.float32
F32R = mybir.dt.float32r
AF = mybir.ActivationFunctionType
OP = mybir.AluOpType

B, T, H, E, S, L = 32, 512, 1024, 2048, 128, 512
NCORES = 8
BPC = B // NCORES          # batch elements per core
EPS = 1e-5
HC = H // 128              # 8 H-chunks
EC = E // 128              # 16 E-chunks
TC = T // 128              # 4 token chunks
NQKV = E + S               # 2176 qkv columns (v cols, then base cols)


def _emit(nc, with_vbias):
    x_d = nc.dram_tensor("x_in", [BPC, T, H], F32, kind="ExternalInput")
    wqkv_d = nc.dram_tensor("wqkv_in", [H, NQKV], F32R, kind="ExternalInput")
    wu_d = nc.dram_tensor("wu_in", [H, E], F32R, kind="ExternalInput")
    wo_d = nc.dram_tensor("wo_in", [E, H], F32R, kind="ExternalInput")
    biasT_d = nc.dram_tensor("biasT_in", [T, T], F32, kind="ExternalInput")
    ropeC_d = nc.dram_tensor("ropeC_in", [S, T], F32, kind="ExternalInput")
    ropeS_d = nc.dram_tensor("ropeS_in", [S, T], F32, kind="ExternalInput")
    gb_d = nc.dram_tensor("gb_in", [S, 4], F32, kind="ExternalInput")
    ubu_d = nc.dram_tensor("ubu_in", [128, EC], F32, kind="ExternalInput")
    ubb_d = nc.dram_tensor("ubb_in", [S, 1], F32, kind="ExternalInput")
    vb_d = nc.dram_tensor("vb_in", [1, E], F32R, kind="ExternalInput")
    y_d = nc.dram_tensor("y_out", [BPC, T, H], F32, kind="ExternalOutput")

    with tile.TileContext(nc) as tc, ExitStack() as ctx:
        consts = ctx.enter_context(tc.tile_pool(name="consts", bufs=1))
        holdp = ctx.enter_context(tc.tile_pool(name="holdp", bufs=1))
        dram = ctx.enter_context(tc.tile_pool(name="dram", bufs=1, space="DRAM"))
        xnT_spill = dram.tile([BPC, HC, 128, T], F32R)
        aT_spill = dram.tile([BPC, EC, 128, T], F32)

        # ================= PHASE 1 =================
        with ExitStack() as p1:
            # Pool allocation order = SBUF stack order = death-time order:
            # xp/xnp die after last LN, p1w after last qkv matmul, so the
            # phase-2 weight loads (which reuse this space) start early.
            xp = p1.enter_context(tc.tile_pool(name="xp", bufs=2))
            xnp = p1.enter_context(tc.tile_pool(name="xnp", bufs=2))
            p1w = p1.enter_context(tc.tile_pool(name="p1w", bufs=1))
            xnTp = p1.enter_context(tc.tile_pool(name="xnTp", bufs=1))
            vp = p1.enter_context(tc.tile_pool(name="vp", bufs=4))
            rw = p1.enter_context(tc.tile_pool(name="rw", bufs=2))
            ktp = p1.enter_context(tc.tile_pool(name="ktp", bufs=4))
            p1c = p1.enter_context(tc.tile_pool(name="p1c", bufs=1))
            atp = p1.enter_context(tc.tile_pool(name="atp", bufs=3 if with_vbias else 5))
            ps = p1.enter_context(tc.tile_pool(name="ps", bufs=4, space="PSUM"))
            psa = p1.enter_context(tc.tile_pool(name="psa", bufs=2, space="PSUM"))
            pstp = p1.enter_context(tc.tile_pool(name="pstp", bufs=2, space="PSUM"))

            eps_t = p1c.tile([128, 1], F32, tag="eps")
            nc.vector.memset(eps_t, EPS)

            def load_x_ln(e):
                xn_tiles = []
                for tci in range(TC):
                    xt = xp.tile([128, H], F32, tag="x")
                    nc.sync.dma_start(
                        out=xt, in_=x_d[e, tci * 128:(tci + 1) * 128, :])
                    st = rw.tile([128, 2, 6], F32, tag="bnst", bufs=4)
                    xv = xt[:].rearrange("p (g d) -> p g d", g=2)
                    nc.vector.bn_stats(out=st[:, 0, :], in_=xv[:, 0, :])
                    nc.vector.bn_stats(out=st[:, 1, :], in_=xv[:, 1, :])
                    mv = rw.tile([128, 2], F32, tag="bnmv", bufs=4)
                    nc.vector.bn_aggr(out=mv[:], in_=st[:])
                    std = rw.tile([128, 1], F32, tag="std", bufs=4)
                    nc.scalar.activation(
                        out=std[:], in_=mv[:, 1:2], func=AF.Sqrt,
                        bias=eps_t[:], scale=1.0)
                    rstd = rw.tile([128, 1], F32, tag="rstd", bufs=4)
                    nc.vector.reciprocal(out=rstd[:], in_=std[:])
                    xn = xnp.tile([128, H], F32R, tag="xn")
                    nc.vector.tensor_scalar(
                        out=xn[:], in0=xt[:], scalar1=mv[:, 0:1],
                        scalar2=rstd[:], op0=OP.subtract, op1=OP.mult)
                    xn_tiles.append(xn)
                return xn_tiles

            def transpose_xn(e, xn_tiles):
                """Per-tc transpose groups: xn(tc) is consumed right away."""
                if e == 0:
                    # element 0's xnT survives into phase 2 (no spill)
                    xnT = holdp.tile([128, HC, T], F32R, tag="xnT_hold",
                                     name="xnT_hold")
                else:
                    xnT = xnTp.tile([128, HC, T], F32R, tag="xnT")
                for tci in range(TC):
                    for hcg in range(2):
                        tps = pstp.tile([128, 4, 128], F32R, tag="tps",
                                      name="tps")
                        for j in range(4):
                            hc = hcg * 4 + j
                            nc.tensor.transpose(
                                tps[:, j, :],
                                xn_tiles[tci][:, hc * 128:(hc + 1) * 128],
                                ident[:])
                        nc.any.tensor_copy(
                            out=xnT[:, hcg * 4:(hcg + 1) * 4,
                                    tci * 128:(tci + 1) * 128],
                            in_=tps[:])
                if e != 0:
                    nc.sync.dma_start(
                        out=xnT_spill[e].rearrange("c p t -> p c t"),
                        in_=xnT[:])
                return xnT

            # ---- priority order: x(e0)+LN first, then consts, weights ----
            xn0 = load_x_ln(0)

            ident_f = p1c.tile([128, 128], F32, tag="ident_f")
            make_identity(nc, ident_f)
            ident = p1c.tile([128, 128], F32R, tag="ident")
            nc.vector.tensor_copy(out=ident[:], in_=ident_f[:])
            perm_f = p1c.tile([128, 128], F32, tag="perm_f")
            nc.gpsimd.memset(perm_f, 0.0)
            for base in (-64, 64):
                nc.gpsimd.affine_select(
                    out=perm_f, in_=perm_f, compare_op=OP.not_equal,
                    fill=1.0, base=base, pattern=[[-1, 128]],
                    channel_multiplier=1)
            perm = p1c.tile([128, 128], F32R, tag="perm")
            nc.vector.tensor_copy(out=perm[:], in_=perm_f[:])
            gb = p1c.tile([S, 4], F32, tag="gb")
            nc.sync.dma_start(out=gb, in_=gb_d[:])
            ubu = consts.tile([128, EC], F32, tag="ubu")
            nc.sync.dma_start(out=ubu, in_=ubu_d[:])
            ubb = p1c.tile([S, 1], F32, tag="ubb")
            nc.sync.dma_start(out=ubb, in_=ubb_d[:])
            if with_vbias:
                of = p1c.tile([1, 128], F32, tag="ones_row_f")
                nc.vector.memset(of, 1.0)
                ones_row = p1c.tile([1, 128], F32R, tag="ones_row")
                nc.vector.tensor_copy(out=ones_row[:], in_=of[:])
                vb_row = p1c.tile([1, E], F32R, tag="vb_row")
                nc.sync.dma_start(out=vb_row, in_=vb_d[:])

            ropeC = p1c.tile([S, T], F32, tag="ropeC")
            nc.sync.dma_start(out=ropeC, in_=ropeC_d[:])
            ropeS = p1c.tile([S, T], F32, tag="ropeS")
            nc.sync.dma_start(out=ropeS, in_=ropeS_d[:])
            biasT = p1c.tile([128, TC, T], F32, tag="biasT")
            nc.sync.dma_start(
                out=biasT, in_=biasT_d[:].rearrange("(c p) i -> p c i", p=128))

            # qkv weights: per-chunk DMAs so elem-0 compute chases arrival
            wqkv = []
            for k in range(HC):
                wqk = p1w.tile([128, NQKV], F32R, tag=f"wqkv{k}", name="wqk")
                nc.sync.dma_start(
                    out=wqk, in_=wqkv_d[k * 128:(k + 1) * 128, :])
                wqkv.append(wqk)

            def rope_pre(bps):
                ubT = rw.tile([S, T], F32, tag="ubT")
                nc.scalar.activation(
                    out=ubT[:], in_=bps[:], func=AF.Silu, bias=ubb[:],
                    scale=1.0)
                pres = []
                for qi in (0, 1):
                    pre = rw.tile([S, T], F32R, tag="pre")
                    nc.vector.tensor_scalar(
                        out=pre[:], in0=ubT[:],
                        scalar1=gb[:, 2 * qi:2 * qi + 1],
                        scalar2=gb[:, 2 * qi + 1:2 * qi + 2],
                        op0=OP.mult, op1=OP.add)
                    pres.append(pre)
                return pres

            def rope_finish(pres):
                qkts = []
                for pre in pres:
                    sps = ps.tile([128, T], F32, tag="ps", name="sps")
                    nc.tensor.matmul(sps[:], perm[:], pre[:],
                                     start=True, stop=True)
                    t1 = rw.tile([S, T], F32, tag="t1")
                    nc.vector.tensor_tensor(
                        out=t1[:], in0=pre[:], in1=ropeC[:], op=OP.mult)
                    t2 = rw.tile([S, T], F32, tag="t2")
                    nc.vector.tensor_tensor(
                        out=t2[:], in0=sps[:], in1=ropeS[:], op=OP.mult)
                    qkt = rw.tile([S, T], F32R, tag="qkt", bufs=2 if with_vbias else 3)
                    nc.vector.tensor_tensor(
                        out=qkt[:], in0=t1[:], in1=t2[:], op=OP.add)
                    qkts.append(qkt)
                return qkts

            def base_proj(xnT):
                bps = ps.tile([128, T], F32, tag="ps", name="bps")
                for k in range(HC):
                    nc.tensor.matmul(
                        bps[:], wqkv[k][:, E:E + S], xnT[:, k, :],
                        start=(k == 0), stop=(k == HC - 1))
                return bps

            def qkv_chase(xnT):
                """Elem-0 path: base + v projections in 6-bank waves that
                consume weight chunks as they arrive."""
                v_tiles = [vp.tile([128, E], F32R, tag="v", name="v")
                           for _ in range(TC)]
                pairs = [(tci, fs) for tci in range(TC)
                         for fs in range(E // 512)]

                def fin(tci, fs, bank):
                    if with_vbias:
                        nc.tensor.matmul(
                            bank[:], ones_row[:],
                            vb_row[:, fs * 512:(fs + 1) * 512],
                            start=False, stop=True)
                    nc.scalar.activation(
                        out=v_tiles[tci][:, fs * 512:(fs + 1) * 512],
                        in_=bank[:], func=AF.Silu)

                bps = ps.tile([128, T], F32, tag="ps", name="bps")
                wave0 = pairs[:5]
                banks0 = {p: ps.tile([128, 512], F32, tag="ps", name="vbank")
                          for p in wave0}
                for k in range(HC):
                    nc.tensor.matmul(
                        bps[:], wqkv[k][:, E:E + S], xnT[:, k, :],
                        start=(k == 0), stop=(k == HC - 1))
                    for (tci, fs) in wave0:
                        nc.tensor.matmul(
                            banks0[(tci, fs)][:],
                            xnT[:, k, tci * 128:(tci + 1) * 128],
                            wqkv[k][:, fs * 512:(fs + 1) * 512],
                            start=(k == 0),
                            stop=(k == HC - 1) and not with_vbias)
                pres = rope_pre(bps)
                for (tci, fs) in wave0:
                    fin(tci, fs, banks0[(tci, fs)])
                qkts = None
                for wn, wi in enumerate(range(5, len(pairs), 6)):
                    wave = pairs[wi:wi + 6]
                    banks = {p: ps.tile([128, 512], F32, tag="ps",
                                        name="vbank") for p in wave}
                    for k in range(HC):
                        for (tci, fs) in wave:
                            nc.tensor.matmul(
                                banks[(tci, fs)][:],
                                xnT[:, k, tci * 128:(tci + 1) * 128],
                                wqkv[k][:, fs * 512:(fs + 1) * 512],
                                start=(k == 0),
                                stop=(k == HC - 1) and not with_vbias)
                    if wn == 0:
                        qkts = rope_finish(pres)
                    for (tci, fs) in wave:
                        fin(tci, fs, banks[(tci, fs)])
                return qkts, v_tiles

            def v_proj(xnT, outer_k, rope_mid=None):
                v_tiles = [vp.tile([128, E], F32R, tag="v", name="v")
                           for _ in range(TC)]
                pairs = [(tci, fs) for tci in range(TC)
                         for fs in range(E // 512)]

                def fin(tci, fs, bank):
                    if with_vbias:
                        nc.tensor.matmul(
                            bank[:], ones_row[:],
                            vb_row[:, fs * 512:(fs + 1) * 512],
                            start=False, stop=True)
                    nc.scalar.activation(
                        out=v_tiles[tci][:, fs * 512:(fs + 1) * 512],
                        in_=bank[:], func=AF.Silu)

                if outer_k:
                    for wi in range(0, len(pairs), 3):
                        wave = pairs[wi:wi + 3]
                        banks = {p: ps.tile([128, 512], F32, tag="ps",
                                            name="vbank") for p in wave}
                        for k in range(HC):
                            for (tci, fs) in wave:
                                nc.tensor.matmul(
                                    banks[(tci, fs)][:],
                                    xnT[:, k, tci * 128:(tci + 1) * 128],
                                    wqkv[k][:, fs * 512:(fs + 1) * 512],
                                    start=(k == 0),
                                    stop=(k == HC - 1) and not with_vbias)
                        for (tci, fs) in wave:
                            fin(tci, fs, banks[(tci, fs)])
                else:
                    for pi, (tci, fs) in enumerate(pairs):
                        bank = ps.tile([128, 512], F32, tag="ps", name="vbank")
                        for k in range(HC):
                            nc.tensor.matmul(
                                bank[:],
                                xnT[:, k, tci * 128:(tci + 1) * 128],
                                wqkv[k][:, fs * 512:(fs + 1) * 512],
                                start=(k == 0),
                                stop=(k == HC - 1) and not with_vbias)
                        fin(tci, fs, bank)
                        if pi == 3 and rope_mid is not None:
                            rope_mid()
                return v_tiles

            def attention(e, qT, kT, v_tiles):
                kern_tiles = []
                for jc in range(TC):
                    scps = ps.tile([128, T], F32, tag="ps", name="scps")
                    nc.tensor.matmul(
                        scps[:], kT[:, jc * 128:(jc + 1) * 128], qT[:],
                        start=True, stop=True)
                    tadd = rw.tile([128, T], F32, tag="tadd", bufs=1 if with_vbias else 2)
                    nc.vector.tensor_tensor(
                        out=tadd[:], in0=scps[:], in1=biasT[:, jc, :],
                        op=OP.add)
                    kt = ktp.tile([128, T], F32R, tag="kern")
                    # relu(t)^2 == max(t,0)*t, fused on DVE
                    nc.vector.scalar_tensor_tensor(
                        out=kt[:], in0=tadd[:], scalar=0.0, in1=tadd[:],
                        op0=OP.max, op1=OP.mult)
                    kern_tiles.append(kt)
                for ec in range(EC):
                    aps = psa.tile([128, T], F32, tag="psa", name="aps")
                    for jc in range(TC):
                        nc.tensor.matmul(
                            aps[:],
                            v_tiles[jc][:, ec * 128:(ec + 1) * 128],
                            kern_tiles[jc][:],
                            start=(jc == 0), stop=(jc == TC - 1))
                    at = atp.tile([128, T], F32, tag="aT")
                    nc.any.tensor_copy(out=at[:], in_=aps[:])
                    nc.sync.dma_start(out=aT_spill[e, ec], in_=at[:])

            xnT_hold = None
            for e in range(BPC):
                xn_e = xn0 if e == 0 else load_x_ln(e)
                xnT_e = transpose_xn(e, xn_e)
                if e == 0:
                    xnT_hold = xnT_e
                if e == 0:
                    qk_e, v_e = qkv_chase(xnT_e)
                else:
                    bps = base_proj(xnT_e)
                    pres = rope_pre(bps)
                    qk_box = []

                    def rope_mid(pres=pres, qk_box=qk_box):
                        qk_box.extend(rope_finish(pres))

                    v_e = v_proj(xnT_e, outer_k=False, rope_mid=rope_mid)
                    qk_e = qk_box
                attention(e, *qk_e, v_e)

        # ================= PHASE 2 =================
        with ExitStack() as p2:
            xnT2p = p2.enter_context(tc.tile_pool(name="xnT2p", bufs=1))
            p2w = p2.enter_context(tc.tile_pool(name="p2w", bufs=1))
            utp = p2.enter_context(tc.tile_pool(name="utp", bufs=1))
            at2p = p2.enter_context(tc.tile_pool(name="at2p", bufs=2))
            gtp = p2.enter_context(tc.tile_pool(name="gtp", bufs=EC))
            yp = p2.enter_context(tc.tile_pool(name="yp", bufs=2))
            xrp = p2.enter_context(tc.tile_pool(name="xrp", bufs=2))
            ps2 = p2.enter_context(tc.tile_pool(name="ps2", bufs=8,
                                                space="PSUM"))

            def load_xnT2(e):
                xnT2 = xnT2p.tile([128, HC, T], F32R, tag="xnT2")
                nc.sync.dma_start(
                    out=xnT2, in_=xnT_spill[e].rearrange("c p t -> p c t"))
                return xnT2

            wu = []
            for k in range(HC):
                wuk = p2w.tile([128, E], F32R, tag=f"wu{k}", name="wuk")
                nc.sync.dma_start(out=wuk, in_=wu_d[k * 128:(k + 1) * 128, :])
                wu.append(wuk)
            wo = [None] * EC

            def load_wo_chunk(ec):
                woc = p2w.tile([128, H], F32R, tag=f"wo{ec}", name="woc")
                nc.sync.dma_start(
                    out=woc, in_=wo_d[ec * 128:(ec + 1) * 128, :])
                wo[ec] = woc

            def gate_one(e, ec, bank):
                ut = utp.tile([128, T], F32, tag="uT")
                nc.scalar.activation(
                    out=ut[:], in_=bank[:], func=AF.Silu,
                    bias=ubu[:, ec:ec + 1], scale=1.0)
                at2 = at2p.tile([128, T], F32, tag="aT2")
                nc.sync.dma_start(out=at2, in_=aT_spill[e, ec])
                gt = gtp.tile([128, T], F32R, tag="gT")
                nc.vector.tensor_tensor(
                    out=gt[:], in0=ut[:], in1=at2[:], op=OP.mult)
                return gt

            def u_proj_gate(e, xnT2, outer_k):
                g_tiles = []
                if outer_k:
                    nwo = 0
                    for wi in range(0, EC, 6):
                        wave = list(range(wi, min(wi + 6, EC)))
                        banks = {ec: ps2.tile([128, T], F32, tag="ps",
                                              name="ubank") for ec in wave}
                        for k in range(HC):
                            for ec in wave:
                                nc.tensor.matmul(
                                    banks[ec][:],
                                    wu[k][:, ec * 128:(ec + 1) * 128],
                                    xnT2[:, k, :],
                                    start=(k == 0), stop=(k == HC - 1))
                        for ec in wave:
                            g_tiles.append(gate_one(e, ec, banks[ec]))
                            if nwo < EC:
                                load_wo_chunk(nwo)
                                nwo += 1
                    while nwo < EC:
                        load_wo_chunk(nwo)
                        nwo += 1
                else:
                    for ec in range(EC):
                        bank = ps2.tile([128, T], F32, tag="ps", name="ubank")
                        for k in range(HC):
                            nc.tensor.matmul(
                                bank[:], wu[k][:, ec * 128:(ec + 1) * 128],
                                xnT2[:, k, :],
                                start=(k == 0), stop=(k == HC - 1))
                        g_tiles.append(gate_one(e, ec, bank))
                return g_tiles

            def o_proj_tail(e, g_tiles):
                # last element: per-(tc,hs) accumulation with immediate
                # evacuation, so the kernel tail isn't one long serial chain
                pairs8 = [(tci, hs) for tci in range(TC)
                          for hs in range(H // 512)]
                for (tci, hs) in pairs8:
                    xr = xrp.tile([128, 512], F32, tag="xr")
                    nc.sync.dma_start(
                        out=xr,
                        in_=x_d[e, tci * 128:(tci + 1) * 128,
                                hs * 512:(hs + 1) * 512])
                    yps = ps2.tile([128, 512], F32, tag="ps", name="yps")
                    for ec in range(EC):
                        nc.tensor.matmul(
                            yps[:],
                            g_tiles[ec][:, tci * 128:(tci + 1) * 128],
                            wo[ec][:, hs * 512:(hs + 1) * 512],
                            start=(ec == 0), stop=(ec == EC - 1))
                    yt = yp.tile([128, 512], F32, tag="y")
                    nc.vector.tensor_tensor(
                        out=yt[:], in0=yps[:], in1=xr[:], op=OP.add)
                    nc.sync.dma_start(
                        out=y_d[e, tci * 128:(tci + 1) * 128,
                                hs * 512:(hs + 1) * 512],
                        in_=yt[:])

            def o_proj(e, g_tiles):
                # outer-ec accumulation into 8 banks: consumes each gT chunk
                # exactly once (frees it for the next element's gating) and
                # chases wo chunk arrival on the first element.
                pairs8 = [(tci, hs) for tci in range(TC)
                          for hs in range(H // 512)]
                banks = {p: ps2.tile([128, 512], F32, tag="ps", name="obank")
                         for p in pairs8}
                xrs = {}
                for (tci, hs) in pairs8:
                    xr = xrp.tile([128, 512], F32, tag="xr")
                    nc.sync.dma_start(
                        out=xr,
                        in_=x_d[e, tci * 128:(tci + 1) * 128,
                                hs * 512:(hs + 1) * 512])
                    xrs[(tci, hs)] = xr
                for ec in range(EC):
                    for (tci, hs) in pairs8:
                        nc.tensor.matmul(
                            banks[(tci, hs)][:],
                            g_tiles[ec][:, tci * 128:(tci + 1) * 128],
                            wo[ec][:, hs * 512:(hs + 1) * 512],
                            start=(ec == 0), stop=(ec == EC - 1))
                for (tci, hs) in pairs8:
                    yt = yp.tile([128, 512], F32, tag="y")
                    nc.vector.tensor_tensor(
                        out=yt[:], in0=banks[(tci, hs)][:], in1=xrs[(tci, hs)][:],
                        op=OP.add)
                    nc.sync.dma_start(
                        out=y_d[e, tci * 128:(tci + 1) * 128,
                                hs * 512:(hs + 1) * 512],
                        in_=yt[:])

            for e in range(BPC):
                xnT2 = xnT_hold if e == 0 else load_xnT2(e)
                g_tiles = u_proj_gate(e, xnT2, outer_k=(e == 0))
                if e == BPC - 1:
                    o_proj_tail(e, g_tiles)
                else:
                    o_proj(e, g_tiles)

    return nc


_BUILD_CACHE = {}


def _get_nc(with_vbias):
    key = bool(with_vbias)
    if key not in _BUILD_CACHE:
        nc = bacc.Bacc("TRN2", target_bir_lowering=False)
        _emit(nc, with_vbias)
        nc.compile()
        _BUILD_CACHE[key] = nc
    return _BUILD_CACHE[key]


def _rope_tables():
    """Rope sin/cos tables, computed with jax-on-cpu float32 ops exactly as
    the reference does (sin/cos of large fp32 arguments are implementation-
    sensitive, so matching op-for-op matters)."""
    import jax
    import jax.numpy as jnp

    cpu = jax.devices("cpu")[0]
    with jax.default_device(cpu):
        half = S // 2
        pos = jnp.arange(T, dtype=jnp.float32)
        inv_freq = 10000.0 ** (jnp.arange(half, dtype=jnp.float32) / half)
        sinusoid = pos[:, None] * inv_freq[None, :]          # [T, half]
        sin = np.asarray(jnp.sin(sinusoid)).astype(np.float32)
        cos = np.asarray(jnp.cos(sinusoid)).astype(np.float32)
    C = np.empty((S, T), np.float32)
    Sg = np.empty((S, T), np.float32)
    C[:half] = cos.T
    C[half:] = cos.T
    Sg[:half] = -sin.T   # q[s<64] = pre[s]*cos - pre[s+64]*sin
    Sg[half:] = sin.T    # q[s>=64] = pre[s]*cos + pre[s-64]*sin
    return C, Sg


def _host_prep(x, ln_w, ln_b, uv_w, uv_b, gamma, beta, w, o_w, o_b):
    w_eff = uv_w * ln_w[None, :]                 # fold ln scale into weights
    uvb_eff = uv_b + uv_w @ ln_b                 # fold ln shift into biases
    uv_wT = np.ascontiguousarray(w_eff.T)        # [H, 2E+S]
    wqkv = np.ascontiguousarray(uv_wT[:, E:])    # [H, E+S]
    wu = np.ascontiguousarray(uv_wT[:, :E])      # [H, E]
    wo = np.ascontiguousarray(o_w.T)             # [E, H]

    idx = np.arange(T)
    biasT = np.ascontiguousarray(w[idx[:, None] - idx[None, :] + (L - 1)])

    ropeC, ropeS = _rope_tables()

    inv_sqrt_s = np.float32(1.0 / np.sqrt(np.float32(S)))
    gb = np.stack([gamma[0] * inv_sqrt_s, beta[0] * inv_sqrt_s,
                   gamma[1], beta[1]], axis=1).astype(np.float32)

    ubu = np.ascontiguousarray(
        uvb_eff[:E].reshape(EC, 128).T).astype(np.float32)
    ubb = uvb_eff[2 * E:].reshape(S, 1).astype(np.float32)
    vb = uvb_eff[E:2 * E].reshape(1, E).astype(np.float32)
    return {
        "wqkv_in": wqkv, "wu_in": wu, "wo_in": wo, "biasT_in": biasT,
        "ropeC_in": ropeC, "ropeS_in": ropeS, "gb_in": gb,
        "ubu_in": ubu, "ubb_in": ubb, "vb_in": vb,
    }


def kernel(x, ln_w, ln_b, uv_w, uv_b, gamma, beta, w, o_w, o_b):
    x = np.ascontiguousarray(np.asarray(x, dtype=np.float32))
    args = [np.asarray(a, np.float32) for a in
            (ln_w, ln_b, uv_w, uv_b, gamma, beta, w, o_w, o_b)]
    ln_w, ln_b, uv_w, uv_b, gamma, beta, w, o_w, o_b = args

    shared = _host_prep(x, ln_w, ln_b, uv_w, uv_b, gamma, beta, w, o_w, o_b)
    with_vbias = bool(np.any(shared["vb_in"]))
    nc = _get_nc(with_vbias)

    in_maps = []
    for c in range(NCORES):
        m = dict(shared)
        m["x_in"] = np.ascontiguousarray(x[c * BPC:(c + 1) * BPC])
        in_maps.append(m)

    res = run_bass_kernel_spmd(nc, in_maps, core_ids=list(range(NCORES)))
    out = np.concatenate([r["y_out"] for r in res.results], axis=0)
    if np.any(o_b):
        out = out + o_b[None, None, :]
    return out


# revision 26
# speedup vs baseline: 252.5515x; 1.0071x over previous
"""GAU (Gated Attention Unit) forward on 8 Trainium2 NeuronCores.

Data-parallel over batch: B=32 -> 4 batch elements per core, every core runs
the identical program on its batch shard with full (replicated) weights.

Per-core schedule (two weight-residency phases so SBUF fits):
  Phase 1 (uv_w.T qkv columns resident):
    LayerNorm -> PE-transpose xn -> qkv projection -> rope (PE half-swap) ->
    scoresT + Toeplitz bias -> relu^2 (fused DVE) -> attn @ v.
    Spills xnT and aT (attention output, transposed) to DRAM scratch.
  Phase 2 (uv_w.T u columns + o_w.T resident):
    u projection -> silu -> gate (u * a) -> output projection + residual.

All matmuls run in float32r (full-rate PE) with fp32 PSUM accumulation.
Weight DMAs are chunked and the first batch element consumes chunks as they
arrive (outer-k accumulation waves). SBUF pools are allocated bottom-up in
death-time order so phase-2 weight loads overlap the phase-1 tail. The
element loop is software-pipelined (LN/transpose of e+1 emitted between
projection and attention of e).
"""

import numpy as np
from contextlib import ExitStack

import concourse.bass as bass
import concourse.tile as tile
from concourse import bacc, mybir
from concourse.bass_utils import run_bass_kernel_spmd
from concourse.masks import make_identity

F32 = mybir.dt.float32
F32R = mybir.dt.float32r
AF = mybir.ActivationFunctionType
OP = mybir.AluOpType

B, T, H, E, S, L = 32, 512, 1024, 2048, 128, 512
NCORES = 8
BPC = B // NCORES          # batch elements per core
EPS = 1e-5
HC = H // 128              # 8 H-chunks
EC = E // 128              # 16 E-chunks
TC = T // 128              # 4 token chunks
NQKV = E + S               # 2176 qkv columns (v cols, then base cols)


def _emit(nc, with_vbias):
    x_d = nc.dram_tensor("x_in", [BPC, T, H], F32, kind="ExternalInput")
    wqkv_d = nc.dram_tensor("wqkv_in", [H, NQKV], F32R, kind="ExternalInput")
    wu_d = nc.dram_tensor("wu_in", [H, E], F32R, kind="ExternalInput")
    wo_d = nc.dram_tensor("wo_in", [E, H], F32R, kind="ExternalInput")
    biasT_d = nc.dram_tensor("biasT_in", [T, T], F32, kind="ExternalInput")
    ropeC_d = nc.dram_tensor("ropeC_in", [S, T], F32, kind="ExternalInput")
    ropeS_d = nc.dram_tensor("ropeS_in", [S, T], F32, kind="ExternalInput")
    gb_d = nc.dram_tensor("gb_in", [S, 4], F32, kind="ExternalInput")
    ubu_d = nc.dram_tensor("ubu_in", [128, EC], F32, kind="ExternalInput")
    ubb_d = nc.dram_tensor("ubb_in", [S, 1], F32, kind="ExternalInput")
    vb_d = nc.dram_tensor("vb_in", [1, E], F32R, kind="ExternalInput")
    y_d = nc.dram_tensor("y_out", [BPC, T, H], F32, kind="ExternalOutput")

    with tile.TileContext(nc) as tc, ExitStack() as ctx:
        consts = ctx.enter_context(tc.tile_pool(name="consts", bufs=1))
        holdp = ctx.enter_context(tc.tile_pool(name="holdp", bufs=1))
        dram = ctx.enter_context(tc.tile_pool(name="dram", bufs=1, space="DRAM"))
        xnT_spill = dram.tile([BPC, HC, 128, T], F32R)
        aT_spill = dram.tile([BPC, EC, 128, T], F32)

        # ================= PHASE 1 =================
        with ExitStack() as p1:
            # Pool allocation order = SBUF stack order = death-time order:
            # xp/xnp die after last LN, p1w after last qkv matmul, so the
            # phase-2 weight loads (which reuse this space) start early.
            xp = p1.enter_context(tc.tile_pool(name="xp", bufs=2))
            xnp = p1.enter_context(tc.tile_pool(name="xnp", bufs=2))
            p1w = p1.enter_context(tc.tile_pool(name="p1w", bufs=1))
            xnTp = p1.enter_context(tc.tile_pool(name="xnTp", bufs=1))
            vp = p1.enter_context(tc.tile_pool(name="vp", bufs=4))
            rw = p1.enter_context(tc.tile_pool(name="rw", bufs=2))
            ktp = p1.enter_context(tc.tile_pool(name="ktp", bufs=4))
            p1c = p1.enter_context(tc.tile_pool(name="p1c", bufs=1))
            atp = p1.enter_context(tc.tile_pool(name="atp", bufs=3 if with_vbias else 5))
            ps = p1.enter_context(tc.tile_pool(name="ps", bufs=4, space="PSUM"))
            psa = p1.enter_context(tc.tile_pool(name="psa", bufs=2, space="PSUM"))
            pstp = p1.enter_context(tc.tile_pool(name="pstp", bufs=2, space="PSUM"))

            eps_t = p1c.tile([128, 1], F32, tag="eps")
            nc.vector.memset(eps_t, EPS)

            def load_x_ln(e):
                xn_tiles = []
                for tci in range(TC):
                    xt = xp.tile([128, H], F32, tag="x")
                    nc.sync.dma_start(
                        out=xt, in_=x_d[e, tci * 128:(tci + 1) * 128, :])
                    st = rw.tile([128, 2, 6], F32, tag="bnst", bufs=4)
                    xv = xt[:].rearrange("p (g d) -> p g d", g=2)
                    nc.vector.bn_stats(out=st[:, 0, :], in_=xv[:, 0, :])
                    nc.vector.bn_stats(out=st[:, 1, :], in_=xv[:, 1, :])
                    mv = rw.tile([128, 2], F32, tag="bnmv", bufs=4)
                    nc.vector.bn_aggr(out=mv[:], in_=st[:])
                    std = rw.tile([128, 1], F32, tag="std", bufs=4)
                    nc.scalar.activation(
                        out=std[:], in_=mv[:, 1:2], func=AF.Sqrt,
                        bias=eps_t[:], scale=1.0)
                    rstd = rw.tile([128, 1], F32, tag="rstd", bufs=4)
                    nc.vector.reciprocal(out=rstd[:], in_=std[:])
                    xn = xnp.tile([128, H], F32R, tag="xn")
                    nc.vector.tensor_scalar(
                        out=xn[:], in0=xt[:], scalar1=mv[:, 0:1],
                        scalar2=rstd[:], op0=OP.subtract, op1=OP.mult)
                    xn_tiles.append(xn)
                return xn_tiles

            def transpose_xn(e, xn_tiles):
                """Per-tc transpose groups: xn(tc) is consumed right away."""
                if e == 0:
                    # element 0's xnT survives into phase 2 (no spill)
                    xnT = holdp.tile([128, HC, T], F32R, tag="xnT_hold",
                                     name="xnT_hold")
                else:
                    xnT = xnTp.tile([128, HC, T], F32R, tag="xnT")
                for tci in range(TC):
                    for hcg in range(2):
                        tps = pstp.tile([128, 4, 128], F32R, tag="tps",
                                      name="tps")
                        for j in range(4):
                            hc = hcg * 4 + j
                            nc.tensor.transpose(
                                tps[:, j, :],
                                xn_tiles[tci][:, hc * 128:(hc + 1) * 128],
                                ident[:])
                        nc.any.tensor_copy(
                            out=xnT[:, hcg * 4:(hcg + 1) * 4,
                                    tci * 128:(tci + 1) * 128],
                            in_=tps[:])
                if e != 0:
                    nc.sync.dma_start(
                        out=xnT_spill[e].rearrange("c p t -> p c t"),
                        in_=xnT[:])
                return xnT

            # ---- priority order: x(e0)+LN first, then consts, weights ----
            xn0 = load_x_ln(0)

            ident_f = p1c.tile([128, 128], F32, tag="ident_f")
            make_identity(nc, ident_f)
            ident = p1c.tile([128, 128], F32R, tag="ident")
            nc.vector.tensor_copy(out=ident[:], in_=ident_f[:])
            perm_f = p1c.tile([128, 128], F32, tag="perm_f")
            nc.gpsimd.memset(perm_f, 0.0)
            for base in (-64, 64):
                nc.gpsimd.affine_select(
                    out=perm_f, in_=perm_f, compare_op=OP.not_equal,
                    fill=1.0, base=base, pattern=[[-1, 128]],
                    channel_multiplier=1)
            perm = p1c.tile([128, 128], F32R, tag="perm")
            nc.vector.tensor_copy(out=perm[:], in_=perm_f[:])
            gb = p1c.tile([S, 4], F32, tag="gb")
            nc.sync.dma_start(out=gb, in_=gb_d[:])
            ubu = consts.tile([128, EC], F32, tag="ubu")
            nc.sync.dma_start(out=ubu, in_=ubu_d[:])
            ubb = p1c.tile([S, 1], F32, tag="ubb")
            nc.sync.dma_start(out=ubb, in_=ubb_d[:])
            if with_vbias:
                of = p1c.tile([1, 128], F32, tag="ones_row_f")
                nc.vector.memset(of, 1.0)
                ones_row = p1c.tile([1, 128], F32R, tag="ones_row")
                nc.vector.tensor_copy(out=ones_row[:], in_=of[:])
                vb_row = p1c.tile([1, E], F32R, tag="vb_row")
                nc.sync.dma_start(out=vb_row, in_=vb_d[:])

            ropeC = p1c.tile([S, T], F32, tag="ropeC")
            nc.sync.dma_start(out=ropeC, in_=ropeC_d[:])
            ropeS = p1c.tile([S, T], F32, tag="ropeS")
            nc.sync.dma_start(out=ropeS, in_=ropeS_d[:])
            biasT = p1c.tile([128, TC, T], F32, tag="biasT")
            nc.sync.dma_start(
                out=biasT, in_=biasT_d[:].rearrange("(c p) i -> p c i", p=128))

            # qkv weights: per-chunk DMAs so elem-0 compute chases arrival
            wqkv = []
            for k in range(HC):
                wqk = p1w.tile([128, NQKV], F32R, tag=f"wqkv{k}", name="wqk")
                nc.sync.dma_start(
                    out=wqk, in_=wqkv_d[k * 128:(k + 1) * 128, :])
                wqkv.append(wqk)

            def rope_pre(bps):
                ubT = rw.tile([S, T], F32, tag="ubT")
                nc.scalar.activation(
                    out=ubT[:], in_=bps[:], func=AF.Silu, bias=ubb[:],
                    scale=1.0)
                pres = []
                for qi in (0, 1):
                    pre = rw.tile([S, T], F32R, tag="pre")
                    nc.vector.tensor_scalar(
                        out=pre[:], in0=ubT[:],
                        scalar1=gb[:, 2 * qi:2 * qi + 1],
                        scalar2=gb[:, 2 * qi + 1:2 * qi + 2],
                        op0=OP.mult, op1=OP.add)
                    pres.append(pre)
                return pres

            def rope_finish(pres):
                qkts = []
                for pre in pres:
                    sps = ps.tile([128, T], F32, tag="ps", name="sps")
                    nc.tensor.matmul(sps[:], perm[:], pre[:],
                                     start=True, stop=True)
                    t1 = rw.tile([S, T], F32, tag="t1")
                    nc.vector.tensor_tensor(
                        out=t1[:], in0=pre[:], in1=ropeC[:], op=OP.mult)
                    t2 = rw.tile([S, T], F32, tag="t2")
                    nc.vector.tensor_tensor(
                        out=t2[:], in0=sps[:], in1=ropeS[:], op=OP.mult)
                    qkt = rw.tile([S, T], F32R, tag="qkt", bufs=2 if with_vbias else 3)
                    nc.vector.tensor_tensor(
                        out=qkt[:], in0=t1[:], in1=t2[:], op=OP.add)
                    qkts.append(qkt)
                return qkts

            def base_proj(xnT):
                bps = ps.tile([128, T], F32, tag="ps", name="bps")
                for k in range(HC):
                    nc.tensor.matmul(
                        bps[:], wqkv[k][:, E:E + S], xnT[:, k, :],
                        start=(k == 0), stop=(k == HC - 1))
                return bps

            def qkv_chase(xnT):
                """Elem-0 path: base + v projections in 6-bank waves that
                consume weight chunks as they arrive."""
                v_tiles = [vp.tile([128, E], F32R, tag="v", name="v")
                           for _ in range(TC)]
                pairs = [(tci, fs) for tci in range(TC)
                         for fs in range(E // 512)]

                def fin(tci, fs, bank):
                    if with_vbias:
                        nc.tensor.matmul(
                            bank[:], ones_row[:],
                            vb_row[:, fs * 512:(fs + 1) * 512],
                            start=False, stop=True)
                    nc.scalar.activation(
                        out=v_tiles[tci][:, fs * 512:(fs + 1) * 512],
                        in_=bank[:], func=AF.Silu)

                bps = ps.tile([128, T], F32, tag="ps", name="bps")
                wave0 = pairs[:5]
                banks0 = {p: (psa.tile([128, 512], F32, tag="psa",
                                       name="vbank") if i < 2 else
                              ps.tile([128, 512], F32, tag="ps",
                                      name="vbank"))
                          for i, p in enumerate(wave0)}
                for k in range(HC):
                    nc.tensor.matmul(
                        bps[:], wqkv[k][:, E:E + S], xnT[:, k, :],
                        start=(k == 0), stop=(k == HC - 1))
                    for (tci, fs) in wave0:
                        nc.tensor.matmul(
                            banks0[(tci, fs)][:],
                            xnT[:, k, tci * 128:(tci + 1) * 128],
                            wqkv[k][:, fs * 512:(fs + 1) * 512],
                            start=(k == 0),
                            stop=(k == HC - 1) and not with_vbias)
                pres = rope_pre(bps)
                for (tci, fs) in wave0:
                    fin(tci, fs, banks0[(tci, fs)])
                qkts = None
                for wn, wi in enumerate(range(5, len(pairs), 6)):
                    wave = pairs[wi:wi + 6]
                    banks = {p: (psa.tile([128, 512], F32, tag="psa",
                                          name="vbank") if i < 2 else
                                 ps.tile([128, 512], F32, tag="ps",
                                         name="vbank"))
                             for i, p in enumerate(wave)}
                    for k in range(HC):
                        for (tci, fs) in wave:
                            nc.tensor.matmul(
                                banks[(tci, fs)][:],
                                xnT[:, k, tci * 128:(tci + 1) * 128],
                                wqkv[k][:, fs * 512:(fs + 1) * 512],
                                start=(k == 0),
                                stop=(k == HC - 1) and not with_vbias)
                    if wn == 0:
                        qkts = rope_finish(pres)
                    for (tci, fs) in wave:
                        fin(tci, fs, banks[(tci, fs)])
                return qkts, v_tiles

            def v_proj(xnT, outer_k, rope_mid=None):
                v_tiles = [vp.tile([128, E], F32R, tag="v", name="v")
                           for _ in range(TC)]
                pairs = [(tci, fs) for tci in range(TC)
                         for fs in range(E // 512)]

                def fin(tci, fs, bank):
                    if with_vbias:
                        nc.tensor.matmul(
                            bank[:], ones_row[:],
                            vb_row[:, fs * 512:(fs + 1) * 512],
                            start=False, stop=True)
                    nc.scalar.activation(
                        out=v_tiles[tci][:, fs * 512:(fs + 1) * 512],
                        in_=bank[:], func=AF.Silu)

                if outer_k:
                    for wi in range(0, len(pairs), 3):
                        wave = pairs[wi:wi + 3]
                        banks = {p: ps.tile([128, 512], F32, tag="ps",
                                            name="vbank") for p in wave}
                        for k in range(HC):
                            for (tci, fs) in wave:
                                nc.tensor.matmul(
                                    banks[(tci, fs)][:],
                                    xnT[:, k, tci * 128:(tci + 1) * 128],
                                    wqkv[k][:, fs * 512:(fs + 1) * 512],
                                    start=(k == 0),
                                    stop=(k == HC - 1) and not with_vbias)
                        for (tci, fs) in wave:
                            fin(tci, fs, banks[(tci, fs)])
                else:
                    for pi, (tci, fs) in enumerate(pairs):
                        bank = ps.tile([128, 512], F32, tag="ps", name="vbank")
                        for k in range(HC):
                            nc.tensor.matmul(
                                bank[:],
                                xnT[:, k, tci * 128:(tci + 1) * 128],
                                wqkv[k][:, fs * 512:(fs + 1) * 512],
                                start=(k == 0),
                                stop=(k == HC - 1) and not with_vbias)
                        fin(tci, fs, bank)
                        if pi == 3 and rope_mid is not None:
                            rope_mid()
                return v_tiles

            def attention(e, qT, kT, v_tiles):
                kern_tiles = []
                for jc in range(TC):
                    scps = ps.tile([128, T], F32, tag="ps", name="scps")
                    nc.tensor.matmul(
                        scps[:], kT[:, jc * 128:(jc + 1) * 128], qT[:],
                        start=True, stop=True)
                    tadd = rw.tile([128, T], F32, tag="tadd", bufs=1 if with_vbias else 2)
                    nc.vector.tensor_tensor(
                        out=tadd[:], in0=scps[:], in1=biasT[:, jc, :],
                        op=OP.add)
                    kt = ktp.tile([128, T], F32R, tag="kern")
                    # relu(t)^2 == max(t,0)*t, fused on DVE
                    nc.vector.scalar_tensor_tensor(
                        out=kt[:], in0=tadd[:], scalar=0.0, in1=tadd[:],
                        op0=OP.max, op1=OP.mult)
                    kern_tiles.append(kt)
                for ec in range(EC):
                    aps = psa.tile([128, T], F32, tag="psa", name="aps")
                    for jc in range(TC):
                        nc.tensor.matmul(
                            aps[:],
                            v_tiles[jc][:, ec * 128:(ec + 1) * 128],
                            kern_tiles[jc][:],
                            start=(jc == 0), stop=(jc == TC - 1))
                    at = atp.tile([128, T], F32, tag="aT")
                    nc.any.tensor_copy(out=at[:], in_=aps[:])
                    nc.sync.dma_start(out=aT_spill[e, ec], in_=at[:])

            xnT_hold = None
            xn_next = xn0
            for e in range(BPC):
                xn_e = xn_next
                xnT_e = transpose_xn(e, xn_e)
                if e == 0:
                    xnT_hold = xnT_e
                if e == 0:
                    qk_e, v_e = qkv_chase(xnT_e)
                else:
                    bps = base_proj(xnT_e)
                    pres = rope_pre(bps)
                    qk_box = []

                    def rope_mid(pres=pres, qk_box=qk_box):
                        qk_box.extend(rope_finish(pres))

                    v_e = v_proj(xnT_e, outer_k=False, rope_mid=rope_mid)
                    qk_e = qk_box
                # prefetch next element's x + LayerNorm ahead of the aT
                # spill burst so its DMAs aren't stuck behind it
                xn_next = load_x_ln(e + 1) if e + 1 < BPC else None
                attention(e, *qk_e, v_e)

        # ================= PHASE 2 =================
        with ExitStack() as p2:
            xnT2p = p2.enter_context(tc.tile_pool(name="xnT2p", bufs=1))
            p2w = p2.enter_context(tc.tile_pool(name="p2w", bufs=1))
            utp = p2.enter_context(tc.tile_pool(name="utp", bufs=1))
            at2p = p2.enter_context(tc.tile_pool(name="at2p", bufs=2))
            gtp = p2.enter_context(tc.tile_pool(name="gtp", bufs=EC))
            yp = p2.enter_context(tc.tile_pool(name="yp", bufs=2))
            xrp = p2.enter_context(tc.tile_pool(name="xrp", bufs=2))
            ps2 = p2.enter_context(tc.tile_pool(name="ps2", bufs=8,
                                                space="PSUM"))

            def load_xnT2(e):
                xnT2 = xnT2p.tile([128, HC, T], F32R, tag="xnT2")
                nc.sync.dma_start(
                    out=xnT2, in_=xnT_spill[e].rearrange("c p t -> p c t"))
                return xnT2

            wu = []
            for k in range(HC):
                wuk = p2w.tile([128, E], F32R, tag=f"wu{k}", name="wuk")
                nc.sync.dma_start(out=wuk, in_=wu_d[k * 128:(k + 1) * 128, :])
                wu.append(wuk)
            wo = [None] * EC

            def load_wo_chunk(ec):
                woc = p2w.tile([128, H], F32R, tag=f"wo{ec}", name="woc")
                nc.sync.dma_start(
                    out=woc, in_=wo_d[ec * 128:(ec + 1) * 128, :])
                wo[ec] = woc

            def gate_one(e, ec, bank):
                ut = utp.tile([128, T], F32, tag="uT")
                nc.scalar.activation(
                    out=ut[:], in_=bank[:], func=AF.Silu,
                    bias=ubu[:, ec:ec + 1], scale=1.0)
                at2 = at2p.tile([128, T], F32, tag="aT2")
                nc.sync.dma_start(out=at2, in_=aT_spill[e, ec])
                gt = gtp.tile([128, T], F32R, tag="gT")
                nc.vector.tensor_tensor(
                    out=gt[:], in0=ut[:], in1=at2[:], op=OP.mult)
                return gt

            def u_proj_gate(e, xnT2, outer_k):
                g_tiles = []
                if outer_k:
                    nwo = 0
                    for wi in range(0, EC, 6):
                        wave = list(range(wi, min(wi + 6, EC)))
                        banks = {ec: ps2.tile([128, T], F32, tag="ps",
                                              name="ubank") for ec in wave}
                        for k in range(HC):
                            for ec in wave:
                                nc.tensor.matmul(
                                    banks[ec][:],
                                    wu[k][:, ec * 128:(ec + 1) * 128],
                                    xnT2[:, k, :],
                                    start=(k == 0), stop=(k == HC - 1))
                        for ec in wave:
                            g_tiles.append(gate_one(e, ec, banks[ec]))
                            if nwo < EC:
                                load_wo_chunk(nwo)
                                nwo += 1
                    while nwo < EC:
                        load_wo_chunk(nwo)
                        nwo += 1
                else:
                    for ec in range(EC):
                        bank = ps2.tile([128, T], F32, tag="ps", name="ubank")
                        for k in range(HC):
                            nc.tensor.matmul(
                                bank[:], wu[k][:, ec * 128:(ec + 1) * 128],
                                xnT2[:, k, :],
                                start=(k == 0), stop=(k == HC - 1))
                        g_tiles.append(gate_one(e, ec, bank))
                return g_tiles

            def o_proj_tail(e, g_tiles):
                # last element: per-(tc,hs) accumulation with immediate
                # evacuation, so the kernel tail isn't one long serial chain
                pairs8 = [(tci, hs) for tci in range(TC)
                          for hs in range(H // 512)]
                for (tci, hs) in pairs8:
                    xr = xrp.tile([128, 512], F32, tag="xr")
                    nc.sync.dma_start(
                        out=xr,
                        in_=x_d[e, tci * 128:(tci + 1) * 128,
                                hs * 512:(hs + 1) * 512])
                    yps = ps2.tile([128, 512], F32, tag="ps", name="yps")
                    for ec in range(EC):
                        nc.tensor.matmul(
                            yps[:],
                            g_tiles[ec][:, tci * 128:(tci + 1) * 128],
                            wo[ec][:, hs * 512:(hs + 1) * 512],
                            start=(ec == 0), stop=(ec == EC - 1))
                    yt = yp.tile([128, 512], F32, tag="y")
                    nc.vector.tensor_tensor(
                        out=yt[:], in0=yps[:], in1=xr[:], op=OP.add)
                    nc.sync.dma_start(
                        out=y_d[e, tci * 128:(tci + 1) * 128,
                                hs * 512:(hs + 1) * 512],
                        in_=yt[:])

            def o_proj(e, g_tiles):
                # outer-ec accumulation into 8 banks: consumes each gT chunk
                # exactly once (frees it for the next element's gating) and
                # chases wo chunk arrival on the first element.
                pairs8 = [(tci, hs) for tci in range(TC)
                          for hs in range(H // 512)]
                banks = {p: ps2.tile([128, 512], F32, tag="ps", name="obank")
                         for p in pairs8}
                xrs = {}
                for (tci, hs) in pairs8:
                    xr = xrp.tile([128, 512], F32, tag="xr")
                    nc.sync.dma_start(
                        out=xr,
                        in_=x_d[e, tci * 128:(tci + 1) * 128,
                                hs * 512:(hs + 1) * 512])
                    xrs[(tci, hs)] = xr
                for ec in range(EC):
                    for (tci, hs) in pairs8:
                        nc.tensor.matmul(
                            banks[(tci, hs)][:],
                            g_tiles[ec][:, tci * 128:(tci + 1) * 128],
                            wo[ec][:, hs * 512:(hs + 1) * 512],
                            start=(ec == 0), stop=(ec == EC - 1))
                for (tci, hs) in pairs8:
                    yt = yp.tile([128, 512], F32, tag="y")
                    nc.vector.tensor_tensor(
                        out=yt[:], in0=banks[(tci, hs)][:], in1=xrs[(tci, hs)][:],
                        op=OP.add)
                    nc.sync.dma_start(
                        out=y_d[e, tci * 128:(tci + 1) * 128,
                                hs * 512:(hs + 1) * 512],
                        in_=yt[:])

            for e in range(BPC):
                xnT2 = xnT_hold if e == 0 else load_xnT2(e)
                g_tiles = u_proj_gate(e, xnT2, outer_k=(e == 0))
                if e == BPC - 1:
                    o_proj_tail(e, g_tiles)
                else:
                    o_proj(e, g_tiles)

    return nc


_BUILD_CACHE = {}


def _get_nc(with_vbias):
    key = bool(with_vbias)
    if key not in _BUILD_CACHE:
        nc = bacc.Bacc("TRN2", target_bir_lowering=False)
        _emit(nc, with_vbias)
        nc.compile()
        _BUILD_CACHE[key] = nc
    return _BUILD_CACHE[key]


def _rope_tables():
    """Rope sin/cos tables, computed with jax-on-cpu float32 ops exactly as
    the reference does (sin/cos of large fp32 arguments are implementation-
    sensitive, so matching op-for-op matters)."""
    import jax
    import jax.numpy as jnp

    cpu = jax.devices("cpu")[0]
    with jax.default_device(cpu):
        half = S // 2
        pos = jnp.arange(T, dtype=jnp.float32)
        inv_freq = 10000.0 ** (jnp.arange(half, dtype=jnp.float32) / half)
        sinusoid = pos[:, None] * inv_freq[None, :]          # [T, half]
        sin = np.asarray(jnp.sin(sinusoid)).astype(np.float32)
        cos = np.asarray(jnp.cos(sinusoid)).astype(np.float32)
    C = np.empty((S, T), np.float32)
    Sg = np.empty((S, T), np.float32)
    C[:half] = cos.T
    C[half:] = cos.T
    Sg[:half] = -sin.T   # q[s<64] = pre[s]*cos - pre[s+64]*sin
    Sg[half:] = sin.T    # q[s>=64] = pre[s]*cos + pre[s-64]*sin
    return C, Sg


def _host_prep(x, ln_w, ln_b, uv_w, uv_b, gamma, beta, w, o_w, o_b):
    w_eff = uv_w * ln_w[None, :]                 # fold ln scale into weights
    uvb_eff = uv_b + uv_w @ ln_b                 # fold ln shift into biases
    uv_wT = np.ascontiguousarray(w_eff.T)        # [H, 2E+S]
    wqkv = np.ascontiguousarray(uv_wT[:, E:])    # [H, E+S]
    wu = np.ascontiguousarray(uv_wT[:, :E])      # [H, E]
    wo = np.ascontiguousarray(o_w.T)             # [E, H]

    idx = np.arange(T)
    biasT = np.ascontiguousarray(w[idx[:, None] - idx[None, :] + (L - 1)])

    ropeC, ropeS = _rope_tables()

    inv_sqrt_s = np.float32(1.0 / np.sqrt(np.float32(S)))
    gb = np.stack([gamma[0] * inv_sqrt_s, beta[0] * inv_sqrt_s,
                   gamma[1], beta[1]], axis=1).astype(np.float32)

    ubu = np.ascontiguousarray(
        uvb_eff[:E].reshape(EC, 128).T).astype(np.float32)
    ubb = uvb_eff[2 * E:].reshape(S, 1).astype(np.float32)
    vb = uvb_eff[E:2 * E].reshape(1, E).astype(np.float32)
    return {
        "wqkv_in": wqkv, "wu_in": wu, "wo_in": wo, "biasT_in": biasT,
        "ropeC_in": ropeC, "ropeS_in": ropeS, "gb_in": gb,
        "ubu_in": ubu, "ubb_in": ubb, "vb_in": vb,
    }


def kernel(x, ln_w, ln_b, uv_w, uv_b, gamma, beta, w, o_w, o_b):
    x = np.ascontiguousarray(np.asarray(x, dtype=np.float32))
    args = [np.asarray(a, np.float32) for a in
            (ln_w, ln_b, uv_w, uv_b, gamma, beta, w, o_w, o_b)]
    ln_w, ln_b, uv_w, uv_b, gamma, beta, w, o_w, o_b = args

    shared = _host_prep(x, ln_w, ln_b, uv_w, uv_b, gamma, beta, w, o_w, o_b)
    with_vbias = bool(np.any(shared["vb_in"]))
    nc = _get_nc(with_vbias)

    in_maps = []
    for c in range(NCORES):
        m = dict(shared)
        m["x_in"] = np.ascontiguousarray(x[c * BPC:(c + 1) * BPC])
        in_maps.append(m)

    res = run_bass_kernel_spmd(nc, in_maps, core_ids=list(range(NCORES)))
    out = np.concatenate([r["y_out"] for r in res.results], axis=0)
    if np.any(o_b):
        out = out + o_b[None, None, :]
    return out
